# revision 8
# baseline (speedup 1.0000x reference)
"""Trainium2 Bass kernel for CausalWanSelfAttention (block-causal + local window + sink).

Strategy (8 NeuronCores, SPMD):
  - Sequence-sharded: core c owns tokens [384c, 384c+384).
  - Each core projects its tokens to Q/K/V (fp32r matmuls), RMS-norm + RoPE
    folded into host-precomputed tables, K/V AllGathered across cores.
  - Attention: the mask is all-or-nothing at 256-token frame granularity, so
    each query block attends a dense context = sink frame (256 tokens, static
    slot) + a 4-slot window of the gathered K/V whose base is derived from
    partition_id at runtime. Invalid/duplicate context rows are suppressed
    with a per-core additive bias (-1e4) fused into the exp activation.
  - Softmax denominator via a ones-column matmul accumulated in PSUM.
  - Output projection runs from the (already transposed) attention output.
"""
import os
import time
import numpy as np

import concourse.bass as bass
import concourse.tile as tile
from concourse import bacc, mybir
from concourse.bass_interp import get_hw_module

F32 = mybir.dt.float32
F32R = mybir.dt.float32r
AF = mybir.ActivationFunctionType
ALU = mybir.AluOpType

DIM = 1536
NH = 12
HD = 128
S = 3072
NC = 8
T = S // NC          # 384 tokens per core
NT = DIM // 128      # 12 o/i tiles
C = HD // 2          # 64 rope pairs
CTX = 1792           # 256 sink + 4*384 window
NCT = CTX // 128     # 14 ctx tiles
NFPB = 3
LOCAL = 6
SINK = 1
EPS = 1e-6
MASK_NEG = -1.0e4

# inputs that are identical on every core (replicated on the device mesh)
REPLICATED = {"wq", "wk", "wv", "wo", "bqr", "bkr", "swapm", "onesc", "onesrow"}

_CACHE = {}
LAST_RESULT = None


def _emit(tc, repeat=1):
    nc = tc.nc
    from contextlib import ExitStack

    xt_d = nc.dram_tensor("xt", [DIM, T], F32, kind="ExternalInput").ap()
    wq_d = nc.dram_tensor("wq", [DIM, DIM], F32, kind="ExternalInput").ap()
    wk_d = nc.dram_tensor("wk", [DIM, DIM], F32, kind="ExternalInput").ap()
    wv_d = nc.dram_tensor("wv", [DIM, DIM], F32, kind="ExternalInput").ap()
    wo_d = nc.dram_tensor("wo", [DIM, DIM], F32, kind="ExternalInput").ap()
    bq_d = nc.dram_tensor("bqr", [128, NT], F32, kind="ExternalInput").ap()
    bk_d = nc.dram_tensor("bkr", [128, NT], F32, kind="ExternalInput").ap()
    qcos_d = nc.dram_tensor("qcos", [DIM, T], F32, kind="ExternalInput").ap()
    qsin_d = nc.dram_tensor("qsin", [DIM, T], F32, kind="ExternalInput").ap()
    kcos_d = nc.dram_tensor("kcos", [DIM, T], F32, kind="ExternalInput").ap()
    ksin_d = nc.dram_tensor("ksin", [DIM, T], F32, kind="ExternalInput").ap()
    swap_d = nc.dram_tensor("swapm", [128, 128], F32, kind="ExternalInput").ap()
    ones_d = nc.dram_tensor("onesc", [128, 1], F32, kind="ExternalInput").ap()
    onesrow_d = nc.dram_tensor("onesrow", [1, 128], F32, kind="ExternalInput").ap()
    mask_d = nc.dram_tensor("maskb", [128, NCT], F32, kind="ExternalInput").ap()
    y_d = nc.dram_tensor("y", [T, DIM], F32, kind="ExternalOutput").ap()

    agk_in = nc.dram_tensor("agk_in", [DIM, T], F32).ap()
    agv_in = nc.dram_tensor("agv_in", [T, DIM], F32).ap()
    agk_out = nc.dram_tensor("agk_out", [NC * DIM, T], F32, addr_space="Shared").ap()
    agv_out = nc.dram_tensor("agv_out", [NC * T, DIM], F32, addr_space="Shared").ap()

    for rep in range(repeat):
        _emit_once(tc, rep, xt_d, wq_d, wk_d, wv_d, wo_d, bq_d, bk_d,
                   qcos_d, qsin_d, kcos_d, ksin_d, swap_d, ones_d, onesrow_d,
                   mask_d, y_d, agk_in, agv_in, agk_out, agv_out)


def _emit_once(tc, rep, xt_d, wq_d, wk_d, wv_d, wo_d, bq_d, bk_d,
               qcos_d, qsin_d, kcos_d, ksin_d, swap_d, ones_d, onesrow_d,
               mask_d, y_d, agk_in, agv_in, agk_out, agv_out):
    nc = tc.nc
    from contextlib import ExitStack
    R = f"_r{rep}"

    with ExitStack() as top:
        wpool = top.enter_context(tc.tile_pool(name="wband" + R, bufs=12))
        persist = top.enter_context(tc.tile_pool(name="persist" + R, bufs=1))

        swap_sb = persist.tile([128, 128], F32R, tag="swap", name="swap" + R)
        nc.gpsimd.dma_start(swap_sb[:], swap_d.bitcast(F32R))
        ones_sb = persist.tile([128, 1], F32R, tag="ones", name="ones" + R)
        nc.gpsimd.dma_start(ones_sb[:], ones_d.bitcast(F32R))
        onesrow_sb = persist.tile([1, 128], F32R, tag="onesrow", name="onesrow" + R)
        nc.gpsimd.dma_start(onesrow_sb[:], onesrow_d.bitcast(F32R))
        mask_sb = persist.tile([128, NCT], F32, tag="mask", name="mask" + R)
        nc.gpsimd.dma_start(mask_sb[:], mask_d)
        bq_sb = persist.tile([128, NT], F32, tag="bq", name="bq" + R)
        nc.gpsimd.dma_start(bq_sb[:], bq_d)
        bk_sb = persist.tile([128, NT], F32, tag="bk", name="bk" + R)
        nc.gpsimd.dma_start(bk_sb[:], bk_d)
        eps_sb = persist.tile([1, 1], F32, tag="eps", name="eps" + R)
        nc.vector.memset(eps_sb[:], EPS)

        qt_rot = [persist.tile([128, T], F32R, tag=f"qtr{n}", name=f"qtr{n}" + R)
                  for n in range(NT)]
        attnT = [persist.tile([128, T], F32R, tag=f"atn{n}", name=f"atn{n}" + R)
                 for n in range(NT)]

        with ExitStack() as ph1:
            xpool = ph1.enter_context(tc.tile_pool(name="xp" + R, bufs=1))
            prepool = ph1.enter_context(tc.tile_pool(name="prep" + R, bufs=1))
            tabpool = ph1.enter_context(tc.tile_pool(name="tabp" + R, bufs=2))
            tmp = ph1.enter_context(tc.tile_pool(name="tmp1" + R, bufs=3))
            pmm = ph1.enter_context(tc.tile_pool(name="pmm" + R, bufs=3, space="PSUM"))
            pss = ph1.enter_context(tc.tile_pool(name="pss" + R, bufs=1, space="PSUM"))
            psw = ph1.enter_context(tc.tile_pool(name="psw" + R, bufs=2, space="PSUM"))
            pbc = ph1.enter_context(tc.tile_pool(name="pbc" + R, bufs=1, space="PSUM"))

            xt_sb = xpool.tile([128, NT, T], F32R, tag="xt", name="xt" + R)
            nc.sync.dma_start(xt_sb[:], xt_d.rearrange("(n d) t -> d n t", n=NT).bitcast(F32R))

            def proj_qk(tagp, w_dram, b_sb, cos_dram, sin_dram, dst_tiles, ag_dst):
                bands = [wpool.tile([128, DIM], F32R, tag="w", name=f"{tagp}wb{_i}" + R)
                         for _i in range(NT)]
                for i in range(NT):
                    nc.scalar.dma_start(bands[i][:], w_dram[128 * i:128 * (i + 1), :].bitcast(F32R))
                ssum_ps = pss.tile([1, T], F32, tag="ss", name=f"{tagp}ss" + R)
                pres = []
                for n in range(NT):
                    ps = pmm.tile([128, T], F32, tag="projps", name=f"{tagp}ps{n}" + R)
                    for i in range(NT):
                        nc.tensor.matmul(ps[:], bands[i][:, 128 * n:128 * (n + 1)],
                                         xt_sb[:, i, :], start=(i == 0), stop=(i == NT - 1))
                    pre = prepool.tile([128, T], F32R, tag=f"pre{n}", name=f"{tagp}pre{n}" + R)
                    nc.vector.tensor_scalar_add(pre[:], ps[:], b_sb[:, n:n + 1])
                    sq = tmp.tile([128, T], F32R, tag="sq", name=f"{tagp}sq{n}" + R)
                    nc.vector.tensor_mul(sq[:], pre[:].bitcast(F32), pre[:].bitcast(F32))
                    nc.tensor.matmul(ssum_ps[:], ones_sb[:], sq[:],
                                     start=(n == 0), stop=(n == NT - 1))
                    pres.append(pre)
                srt = tmp.tile([1, T], F32, tag="srt", name=f"{tagp}srt" + R)
                nc.scalar.activation(srt[:], ssum_ps[:], AF.Sqrt,
                                     bias=eps_sb[:], scale=1.0 / DIM)
                rd = tmp.tile([1, T], F32, tag="rd", name=f"{tagp}rd" + R)
                nc.vector.reciprocal(rd[:], srt[:])
                rd_r = tmp.tile([1, T], F32R, tag="rdr", name=f"{tagp}rdr" + R)
                nc.vector.tensor_copy(rd_r[:], rd[:])
                rd_b = pbc.tile([128, T], F32, tag="rdb", name=f"{tagp}rdb" + R)
                nc.tensor.matmul(rd_b[:], onesrow_sb[:], rd_r[:], start=True, stop=True)
                for n in range(NT):
                    sw_ps = psw.tile([128, T], F32, tag="swp", name=f"{tagp}swp{n}" + R)
                    nc.tensor.matmul(sw_ps[:], swap_sb[:], pres[n][:], start=True, stop=True)
                    cos_t = tabpool.tile([128, T], F32, tag="cost", name=f"{tagp}cos{n}" + R)
                    nc.gpsimd.dma_start(cos_t[:], cos_dram[128 * n:128 * (n + 1), :])
                    sin_t = tabpool.tile([128, T], F32, tag="sint", name=f"{tagp}sin{n}" + R)
                    nc.gpsimd.dma_start(sin_t[:], sin_dram[128 * n:128 * (n + 1), :])
                    m1 = tmp.tile([128, T], F32, tag="m1", name=f"{tagp}m1_{n}" + R)
                    nc.vector.tensor_mul(m1[:], pres[n][:].bitcast(F32), cos_t[:])
                    m2 = tmp.tile([128, T], F32, tag="m2", name=f"{tagp}m2_{n}" + R)
                    nc.vector.tensor_mul(m2[:], sw_ps[:], sin_t[:])
                    m3 = tmp.tile([128, T], F32, tag="m3", name=f"{tagp}m3_{n}" + R)
                    nc.vector.tensor_add(m3[:], m1[:], m2[:])
                    nc.vector.tensor_mul(dst_tiles[n][:], m3[:], rd_b[:])
                    if ag_dst is not None:
                        nc.sync.dma_start(
                            ag_dst.rearrange("(n d) t -> d n t", n=NT)[:, n, :].bitcast(F32R),
                            dst_tiles[n][:])

            # K projection -> agk_in -> AllGather
            kpool = ph1.enter_context(tc.tile_pool(name="kdstp" + R, bufs=3))
            kdst = [kpool.tile([128, T], F32R, tag="kd", name=f"kd{_i}" + R)
                    for _i in range(NT)]
            proj_qk("k", wk_d, bk_sb, kcos_d, ksin_d, kdst, agk_in)
            nc.gpsimd.collective_compute(
                "AllGather", mybir.AluOpType.bypass,
                ins=[agk_in], outs=[agk_out], replica_groups=[list(range(NC))])

            # V projection (natural layout) -> agv_in -> AllGather
            vbands = [wpool.tile([128, DIM], F32R, tag="w", name=f"vb{_i}" + R)
                      for _i in range(NT)]
            for i in range(NT):
                nc.scalar.dma_start(vbands[i][:], wv_d[128 * i:128 * (i + 1), :].bitcast(F32R))
            for tc_i in range(3):
                for oc in range(3):
                    ps = pmm.tile([128, 512], F32, tag="projps", name=f"vps{tc_i}_{oc}" + R)
                    for i in range(NT):
                        nc.tensor.matmul(ps[:], xt_sb[:, i, 128 * tc_i:128 * (tc_i + 1)],
                                         vbands[i][:, 512 * oc:512 * (oc + 1)],
                                         start=(i == 0), stop=(i == NT - 1))
                    vsb = tmp.tile([128, 512], F32R, tag="vsb", name=f"vsb{tc_i}_{oc}" + R)
                    nc.vector.tensor_copy(vsb[:], ps[:])
                    nc.sync.dma_start(
                        agv_in[128 * tc_i:128 * (tc_i + 1), 512 * oc:512 * (oc + 1)].bitcast(F32R),
                        vsb[:])
            nc.gpsimd.collective_compute(
                "AllGather", mybir.AluOpType.bypass,
                ins=[agv_in], outs=[agv_out], replica_groups=[list(range(NC))])

            # Q projection (stays in SBUF, overlaps the collectives)
            proj_qk("q", wq_d, bq_sb, qcos_d, qsin_d, qt_rot, None)

        # --- window base (slots) from partition id: 2*(pid>=4) + 2*(pid>=6)
        e = nc.sync
        pid = e.partition_id()
        r1 = e.alloc_register("wge4" + R)
        e.reg_alu(r1, pid, 3, ALU.subtract)
        e.reg_alu(r1, r1, 0, ALU.max)
        e.reg_alu(r1, r1, 1, ALU.min)
        r2 = e.alloc_register("wge6" + R)
        e.reg_alu(r2, pid, 5, ALU.subtract)
        e.reg_alu(r2, r2, 0, ALU.max)
        e.reg_alu(r2, r2, 1, ALU.min)
        e.reg_alu(r1, r1, r2, ALU.add)
        e.reg_alu(r1, r1, 2, ALU.mult)
        w_sv = e.snap(r1, donate=True, min_val=0, max_val=4)

        # --- attention
        with ExitStack() as ph2:
            apool = ph2.enter_context(tc.tile_pool(name="attnp" + R, bufs=2))
            prp = ph2.enter_context(tc.tile_pool(name="probs" + R, bufs=4))
            rdp = ph2.enter_context(tc.tile_pool(name="rdp" + R, bufs=2))
            outp = ph2.enter_context(tc.tile_pool(name="outp" + R, bufs=3))
            ps_s_pool = ph2.enter_context(tc.tile_pool(name="pss2" + R, bufs=2, space="PSUM"))
            ps_b_pool = ph2.enter_context(tc.tile_pool(name="psb" + R, bufs=1, space="PSUM"))
            ps_o_pool = ph2.enter_context(tc.tile_pool(name="pso" + R, bufs=2, space="PSUM"))
            ps_d_pool = ph2.enter_context(tc.tile_pool(name="psd" + R, bufs=1, space="PSUM"))
            ps_y_pool = ph2.enter_context(tc.tile_pool(name="psy" + R, bufs=2, space="PSUM"))

            # prefetch Wo bands (overlap with attention)
            obands = [wpool.tile([128, DIM], F32R, tag="w", name=f"owb{_i}" + R)
                      for _i in range(NT)]
            for i in range(NT):
                nc.scalar.dma_start(obands[i][:], wo_d[128 * i:128 * (i + 1), :].bitcast(F32R))

            agk4 = agk_out.rearrange("(r n d) t -> d r n t", r=NC, n=NT).bitcast(F32R)
            agv4 = agv_out.rearrange("(r b p) o -> p r b o", r=NC, b=3).bitcast(F32R)

            for h in range(NH):
                kt = apool.tile([128, CTX], F32R, tag="kt", name=f"kt{h}" + R)
                nc.sync.dma_start(kt[:, 0:256], agk4[:, 0, h, 0:256])
                nc.sync.dma_start(kt[:, 256:CTX].rearrange("p (r t) -> p r t", r=4),
                                  agk4[:, bass.ds(w_sv, 4), h, :])
                vt = apool.tile([128, NCT, 128], F32R, tag="vt", name=f"vt{h}" + R)
                nc.sync.dma_start(vt[:, 0:2, :], agv4[:, 0, 0:2, 128 * h:128 * (h + 1)])
                nc.sync.dma_start(vt[:, 2:NCT, :].rearrange("p (r b) o -> p r b o", r=4),
                                  agv4[:, bass.ds(w_sv, 4), :, 128 * h:128 * (h + 1)])

                ps_o = ps_o_pool.tile([128, T], F32, tag="o", name=f"pso{h}" + R)
                ps_d = ps_d_pool.tile([1, T], F32, tag="d", name=f"psd{h}" + R)
                for ct in range(NCT):
                    ps_s = ps_s_pool.tile([128, T], F32, tag="s", name=f"s{h}_{ct}" + R)
                    nc.tensor.matmul(ps_s[:], kt[:, 128 * ct:128 * (ct + 1)], qt_rot[h][:],
                                     start=True, stop=True)
                    pr = prp.tile([128, T], F32R, tag="pr", name=f"pr{h}_{ct}" + R)
                    nc.scalar.activation(pr[:], ps_s[:], AF.Exp,
                                         bias=mask_sb[:, ct:ct + 1], scale=1.0)
                    nc.tensor.matmul(ps_o[:], vt[:, ct, :], pr[:],
                                     start=(ct == 0), stop=(ct == NCT - 1))
                    nc.tensor.matmul(ps_d[:], ones_sb[:], pr[:],
                                     start=(ct == 0), stop=(ct == NCT - 1))
                rd = rdp.tile([1, T], F32, tag="rd2", name=f"rda{h}" + R)
                nc.vector.reciprocal(rd[:], ps_d[:])
                rd_r = rdp.tile([1, T], F32R, tag="rdr2", name=f"rdra{h}" + R)
                nc.vector.tensor_copy(rd_r[:], rd[:])
                rd_b = ps_b_pool.tile([128, T], F32, tag="rdb2", name=f"rdba{h}" + R)
                nc.tensor.matmul(rd_b[:], onesrow_sb[:], rd_r[:], start=True, stop=True)
                rd_bs = rdp.tile([128, T], F32, tag="rdbs", name=f"rdbs{h}" + R)
                nc.vector.tensor_copy(rd_bs[:], rd_b[:])
                nc.vector.tensor_mul(attnT[h][:], ps_o[:], rd_bs[:])

            # --- output projection: y[t, o] = sum_i attnT[i][t] * woT[i, o]
            for tc_i in range(3):
                for oc in range(3):
                    ps = ps_y_pool.tile([128, 512], F32, tag="y", name=f"yps{tc_i}_{oc}" + R)
                    for i in range(NT):
                        nc.tensor.matmul(ps[:], attnT[i][:, 128 * tc_i:128 * (tc_i + 1)],
                                         obands[i][:, 512 * oc:512 * (oc + 1)],
                                         start=(i == 0), stop=(i == NT - 1))
                    osb = outp.tile([128, 512], F32, tag="ob", name=f"osb{tc_i}_{oc}" + R)
                    nc.vector.tensor_copy(osb[:], ps[:])
                    nc.scalar.dma_start(y_d[128 * tc_i:128 * (tc_i + 1), 512 * oc:512 * (oc + 1)],
                                      osb[:])


def _build(repeat=1):
    key = ("nc", repeat)
    if key in _CACHE:
        return _CACHE[key]
    nc = bacc.Bacc("TRN2", target_bir_lowering=False, debug=False,
                   enable_asserts=False, num_devices=NC)
    with tile.TileContext(nc) as tc:
        _emit(tc, repeat)
    nc.compile()
    nc.m = get_hw_module(nc.m)
    _CACHE[key] = nc
    return nc


# ---------------------------------------------------------------------------
# host-side input preparation
# ---------------------------------------------------------------------------

def _pos_table(tab, f, h, w):
    cf = C - 2 * (C // 3)
    ch = C // 3
    tf = np.broadcast_to(tab[:f, :cf][:, None, None, :], (f, h, w, cf))
    th = np.broadcast_to(tab[:h, cf:cf + ch][None, :, None, :], (f, h, w, ch))
    tw = np.broadcast_to(tab[:w, cf + ch:][None, None, :, :], (f, h, w, ch))
    return np.concatenate([tf, th, tw], axis=-1).reshape(f * h * w, C)


def _rope_tables(cosP, sinP, g, scale):
    """(cosT, sinT) [S, DIM] folding g and the score scale.

    Device computes: rot = pre*cosT + swap(pre)*sinT, where swap exchanges
    even/odd partners. Equivalent to scale * rope(g * pre)."""
    cosE = np.repeat(cosP, 2, axis=1)          # [S, HD]
    sinE = np.repeat(sinP, 2, axis=1)
    cosT = np.empty((S, DIM), np.float32)
    sinT = np.empty((S, DIM), np.float32)
    for n in range(NH):
        gh = g[128 * n:128 * (n + 1)]
        cosT[:, 128 * n:128 * (n + 1)] = cosE * gh[None, :] * scale
        sh = np.empty(HD, np.float32)
        sh[0::2] = -gh[1::2]
        sh[1::2] = gh[0::2]
        sinT[:, 128 * n:128 * (n + 1)] = sinE * sh[None, :] * scale
    return cosT, sinT


def _mask_for_core(c):
    qb = c // 2
    frame = np.arange(S) // 256
    blk = frame // NFPB

    def allowed(k):
        return (blk[k] <= qb) & (((qb - blk[k]) * NFPB < LOCAL) | (frame[k] < SINK))

    m = np.full(CTX, MASK_NEG, np.float32)
    if qb >= 2:
        m[0:256] = 0.0
    wbase = 2 * max(qb - 1, 0)
    tok = np.arange(T * wbase, T * wbase + 1536)
    m[256:] = np.where(allowed(tok), 0.0, MASK_NEG)
    return np.ascontiguousarray(m.reshape(NCT, 128).T)  # [128, NCT]


def _prep_in_maps(x, Wq, bq, Wk, bk, Wv, bv, Wo, bo, gq, gk, freqs_cos, freqs_sin, f, h, w):
    x = np.asarray(x, np.float32)
    f, h, w = int(f), int(h), int(w)
    cosP = _pos_table(np.asarray(freqs_cos, np.float32), f, h, w)
    sinP = _pos_table(np.asarray(freqs_sin, np.float32), f, h, w)

    qcosT, qsinT = _rope_tables(cosP, sinP, np.asarray(gq, np.float32), HD ** -0.5)
    kcosT, ksinT = _rope_tables(cosP, sinP, np.asarray(gk, np.float32), 1.0)

    wq_t = np.ascontiguousarray(np.asarray(Wq, np.float32).T)
    wk_t = np.ascontiguousarray(np.asarray(Wk, np.float32).T)
    wv_t = np.ascontiguousarray(np.asarray(Wv, np.float32).T)
    wo_t = np.ascontiguousarray(np.asarray(Wo, np.float32).T)
    bq_r = np.ascontiguousarray(np.asarray(bq, np.float32).reshape(NT, 128).T)
    bk_r = np.ascontiguousarray(np.asarray(bk, np.float32).reshape(NT, 128).T)

    swapm = np.zeros((128, 128), np.float32)
    idx = np.arange(128)
    swapm[idx, idx ^ 1] = 1.0
    onesc = np.ones((128, 1), np.float32)
    onesrow = np.ones((1, 128), np.float32)

    xs = x[0]  # [S, DIM]
    in_maps = []
    for c in range(NC):
        xt_c = np.ascontiguousarray(xs[T * c:T * (c + 1), :].T)
        sl = slice(T * c, T * (c + 1))
        in_maps.append(dict(
            xt=xt_c, wq=wq_t, wk=wk_t, wv=wv_t, wo=wo_t,
            bqr=bq_r, bkr=bk_r,
            qcos=np.ascontiguousarray(qcosT[sl].T), qsin=np.ascontiguousarray(qsinT[sl].T),
            kcos=np.ascontiguousarray(kcosT[sl].T), ksin=np.ascontiguousarray(ksinT[sl].T),
            swapm=swapm, onesc=onesc, onesrow=onesrow, maskb=_mask_for_core(c),
        ))

    bo_eff = np.asarray(bo, np.float32) + np.asarray(bv, np.float32) @ np.asarray(Wo, np.float32).T
    return in_maps, bo_eff


def _assemble(per_core_y, bo_eff):
    out = np.concatenate(per_core_y, axis=0)  # [S, DIM]
    out = out + bo_eff[None, :]
    return out[None].astype(np.float32)


# ---------------------------------------------------------------------------
# execution (PJRT shard_map; replicated specs for weights)
# ---------------------------------------------------------------------------

def _make_runner(nc):
    import jax
    from jax.sharding import Mesh, PartitionSpec
    try:
        from jax.experimental.shard_map import shard_map
    except ImportError:
        from jax.shard_map import shard_map
    from concourse.bass2jax import _bass_exec_p, install_neuronx_cc_hook, partition_id_tensor

    install_neuronx_cc_hook()
    partition_name = nc.partition_id_tensor.name if nc.partition_id_tensor else None
    in_names, out_names, out_avals = [], [], []
    for alloc in nc.m.functions[0].allocations:
        if not isinstance(alloc, mybir.MemoryLocationSet):
            continue
        name = alloc.memorylocations[0].name
        if alloc.kind == "ExternalInput":
            if name != partition_name:
                in_names.append(name)
        elif alloc.kind == "ExternalOutput":
            out_names.append(name)
            out_avals.append(jax.core.ShapedArray(tuple(alloc.tensor_shape),
                                                  mybir.dt.np(alloc.dtype)))
    n_params = len(in_names)
    all_in_names = list(in_names) + out_names
    if partition_name is not None:
        all_in_names.append(partition_name)

    def _body(*args):
        ins = list(args[:n_params])
        zouts = list(args[n_params:])
        extra = [partition_id_tensor()] if partition_name is not None else []
        outs = _bass_exec_p.bind(
            *ins, *zouts, *extra,
            out_avals=tuple(out_avals),
            in_names=tuple(all_in_names),
            out_names=tuple(out_names),
            lowering_input_output_aliases=(),
            sim_require_finite=False,
            sim_require_nnan=False,
            nc=nc,
        )
        return tuple(outs)

    import numpy as _np
    devices = jax.devices()[:NC]
    mesh = Mesh(_np.asarray(devices), ("core",))
    in_specs = tuple(
        PartitionSpec() if name in REPLICATED else PartitionSpec("core")
        for name in in_names
    ) + (PartitionSpec("core"),) * len(out_names)
    out_specs = (PartitionSpec("core"),) * len(out_names)
    fn = jax.jit(shard_map(_body, mesh=mesh, in_specs=in_specs,
                           out_specs=out_specs, check_rep=False))
    return fn, in_names, out_names, out_avals


def _prepare_args(in_maps, in_names, out_avals):
    import jax
    args = []
    for i, name in enumerate(in_names):
        if name in REPLICATED:
            args.append(in_maps[0][name])
        else:
            args.append(np.concatenate([np.asarray(m[name]) for m in in_maps], axis=0))
    for a in out_avals:
        args.append(np.zeros((NC * a.shape[0], *a.shape[1:]), a.dtype))
    return [jax.device_put(a) for a in args]


def _run(nc, in_maps):
    import jax
    key = ("runner", id(nc))
    if key not in _CACHE:
        _CACHE[key] = _make_runner(nc)
    fn, in_names, out_names, out_avals = _CACHE[key]
    args = _prepare_args(in_maps, in_names, out_avals)
    outs = fn(*args)
    jax.block_until_ready(outs)
    results = []
    for c in range(NC):
        r = {}
        for i, name in enumerate(out_names):
            r[name] = np.asarray(outs[i]).reshape(NC, *out_avals[i].shape)[c]
        results.append(r)
    return results


def kernel(**inputs):
    global LAST_RESULT
    in_maps, bo_eff = _prep_in_maps(**inputs)
    nc = _build()
    results = _run(nc, in_maps)
    LAST_RESULT = results
    return _assemble([results[c]["y"] for c in range(NC)], bo_eff)
